# revision 64
# baseline (speedup 1.0000x reference)
"""Multi-head attention (B=2, L=2048, D=1024, H=16) on 8 trn2 cores.

Sharding: core c -> (batch b = c//4) x (head-group hg = c%4, 4 heads each).
W_q/W_k/W_v are column-split, W_o row-split; the 4 partial outputs per
batch are summed on the host (plus bo).

Key compaction: the mask is key-only ([B,1,Lk]), so masked keys are
dropped on the host before upload.  K/V projections, scores, exp and
attn@V all run on the compacted LkP keys (padded to a multiple of 128;
pad slots carry a -30000 bias so exp underflows to exactly 0).

All matmul operands are bf16 (PSUM accumulation stays fp32).  Engine
budget: ACT runs only the exp activations, DVE does every PSUM
evacuation + softmax normalization, PE does matmuls.  The kernel is a
software pipeline over the four 512-wide Lq banks: bank lb's exp-gated
score/attnV loop absorbs, as background PE/DVE ops, the straggler K/V
projection tiles, the Q projection of bank lb+1, the softmax
normalization of bank lb-1 and the output projection of bank lb-2, so
no engine ever sits behind the in-order DVE queue.

  K proj   KT[128,CT,LkP] = (Wk chunk).T @ xkT    (dc-inner, psum rotate)
  V proj   v_all[Lk,h,65] = (xvT chunk).T @ Wv    (bias via K=1 opener mm;
                                                   col 64 = ones for denom)
  scores   ST[128,512]    = KT_c.T @ QT_b         (K = dk = 64)
  exp      P = Exp(ST/sqrt(dk) + mask_bias)       (ACT, one op per tile)
  attn@V   OT[65,512]    += v_all_c.T @ P         (row 64 = denominator)
  evac     otc = OT (bf16)                        (frees the psum bank)
  norm     ot_n = otc[0:64] * bcast(1/otc[64])    (DVE recip + PE ones-mm)
  out proj out[t,half]    = sum_g ot_n[g,t].T @ Wo[g,half] -> bf16 -> DRAM

Zero "primer" matmuls open psum accumulation groups so real matmuls never
carry more than one hardware sync-wait (walrus single-wait budget).
"""

import math
import sys

for _p in ("/opt/trn_rl_repo",):
    if _p not in sys.path:
        sys.path.insert(0, _p)

import numpy as np

import concourse.bass as bass
import concourse.mybir as mybir
import concourse.tile as tile
from concourse import bacc
from concourse.bass import ts
from concourse.bass_utils import run_bass_kernel_spmd
from concourse.tile_rust import add_dep_helper

F32 = mybir.dt.float32

D_MODEL = 1024
NUM_HEADS = 16
D_K = 64
B = 2
L = 2048
N_CORES = 8
HPC = NUM_HEADS // 4  # heads per core (4)
C = HPC * D_K         # attention columns per core (256)
CT = C // 128         # col tile groups (2)
DC = D_MODEL // 128   # d_model chunks (8)
LB = L // 512         # Lq banks (4)
SCALE = float(np.sqrt(D_K))
MASK_BIAS = -30000.0


def build_nc(LkP, L=L, D=D_MODEL, mm_dtype=mybir.dt.bfloat16,
             no_bias=False):
    """Per-core Bass program (SPMD, 8 cores) for LkP compacted keys.
    no_bias=True specializes away the (all-zero) projection biases."""
    DK = D_K
    LTk = LkP // 128          # key tiles
    KB = (LkP + 511) // 512   # xk/xv 512-wide load blocks
    VF = min(2, LTk)          # V tiles projected in the foreground
    MDT = mm_dtype

    nc = bacc.Bacc("TRN2", target_bir_lowering=False, debug=False,
                   num_devices=N_CORES)

    xqT = nc.dram_tensor("xqT", [D, L], MDT, kind="ExternalInput").ap()
    xkT = nc.dram_tensor("xkT", [D, LkP], MDT, kind="ExternalInput").ap()
    xvT = nc.dram_tensor("xvT", [D, LkP], MDT, kind="ExternalInput").ap()
    w = {n: nc.dram_tensor(f"w{n}", [D, C], MDT, kind="ExternalInput").ap()
         for n in ("q", "k", "v")}
    wo = nc.dram_tensor("wo", [C, D], MDT, kind="ExternalInput").ap()
    if not no_bias:
        bias = {n: nc.dram_tensor(f"b{n}", [C], F32,
                                  kind="ExternalInput").ap()
                for n in ("q", "k")}
        bvr = nc.dram_tensor("bvr", [1, C], MDT, kind="ExternalInput").ap()
    mb = nc.dram_tensor("mb", [128, LTk], F32, kind="ExternalInput").ap()
    out = nc.dram_tensor("partial", [L, D], MDT, kind="ExternalOutput").ap()

    with tile.TileContext(nc) as tc:
        with (
            tc.tile_pool(name="consts", bufs=1) as consts,
            tc.tile_pool(name="persist", bufs=1) as persist,
            tc.tile_pool(name="xch", bufs=4) as xch,
            tc.tile_pool(name="work", bufs=4) as work,
            tc.tile_pool(name="norm", bufs=4) as normp,
            tc.tile_pool(name="ostg", bufs=8) as ostgp,
            tc.tile_pool(name="ps", bufs=8, space="PSUM") as psp,
        ):
            def ps_tile(name=None):
                return psp.tile([128, 512], F32, tag="ps", name=name or "pst")

            # ---- constants ----
            # Emitted in deadline order: wk feeds the very first matmuls.
            w_sb, b_sb = {}, {}
            last_dma = [None]

            def chain(xd):
                if last_dma[0] is not None:
                    add_dep_helper(xd.ins, last_dma[0].ins, sync=False,
                                   reason="dma-order")
                last_dma[0] = xd

            def load_w(n):
                w_sb[n] = consts.tile([128, DC, C], MDT, tag=f"w{n}",
                                      name=f"w{n}_sb")
                chain(nc.sync.dma_start(
                    out=w_sb[n],
                    in_=w[n].rearrange("(c p) n -> p c n", p=128)))

            def load_b(n):
                b_sb[n] = consts.tile([128, CT], F32, tag=f"b{n}",
                                      name=f"b{n}_sb")
                nc.gpsimd.dma_start(
                    out=b_sb[n], in_=bias[n].rearrange("(t p) -> p t", p=128))

            # zeros / ones constants are DVE memsets, not DMAs: the SWDGE
            # path costs >1us per descriptor batch on the Pool engine, and
            # DVE-written tiles need no PE primer (one monotonic DVE sem)
            ones_sb = consts.tile([1, 128], MDT, tag="ones")
            nc.vector.memset(ones_sb, 1.0)
            v_all = persist.tile([128, LTk, HPC, DK + 1], MDT, tag="vall")
            nc.vector.memset(v_all[:, :, :, DK], 1.0)

            # small consts ride SWDGE (Pool); the big weights go on the
            # same HWDGE ring as the x blocks, interleaved in deadline
            # order, because the sim's DMA engines drain one queue of
            # transfers — a late weight would sit behind 1MB x blocks
            if not no_bias:
                bvr_sb = consts.tile([1, C], MDT, tag="bvr")
                nc.gpsimd.dma_start(out=bvr_sb, in_=bvr)
                load_b("k")
                load_b("q")
            mb_sb = consts.tile([128, LTk], F32, tag="mb")
            nc.gpsimd.dma_start(out=mb_sb, in_=mb)

            def w_rs(n):
                return w_sb[n].rearrange("p c n -> p (c n)")

            # ---- HWDGE ring: weights + x blocks in deadline order ----
            def xload(src, j, wdt, xb=None, o=0):
                if xb is None:
                    xb = xch.tile([128, DC, 512], MDT, tag="xb", name="xb")
                chain(nc.sync.dma_start(
                    out=xb[:, :, o:o + wdt],
                    in_=src[:, j * 512 + o:j * 512 + o + wdt].rearrange(
                        "(c p) n -> p c n", p=128)))
                return xb

            kw = [min(512, LkP - j * 512) for j in range(KB)]
            xkb, xvb, xqb = [None] * KB, [None] * KB, [None] * LB
            load_w("k")
            # first K block lands as two separate half tiles so the first
            # projection only waits ~0.7us of bytes (a half-written shared
            # tile would make subtile deps wait for both halves)
            k0h = min(256, kw[0])
            xk_seg = []
            t0 = xch.tile([128, DC, 256], MDT, tag="xbh", bufs=2,
                          name="xbh")
            chain(nc.sync.dma_start(
                out=t0[:, :, :k0h],
                in_=xkT[:, 0:k0h].rearrange("(c p) n -> p c n", p=128)))
            xk_seg.append((t0, 0, k0h))
            if kw[0] > k0h:
                t1 = xch.tile([128, DC, 256], MDT, tag="xbh", bufs=2,
                              name="xbh")
                chain(nc.sync.dma_start(
                    out=t1[:, :, :kw[0] - k0h],
                    in_=xkT[:, k0h:kw[0]].rearrange(
                        "(c p) n -> p c n", p=128)))
                xk_seg.append((t1, k0h, kw[0] - k0h))
            xqb[0] = xload(xqT, 0, 512)
            load_w("q")
            load_w("v")
            xvb[0] = xload(xvT, 0, kw[0])
            for j in range(1, KB):
                xkb[j] = xload(xkT, j, kw[j])
                xvb[j] = xload(xvT, j, kw[j])
            wo_sb = consts.tile([128, CT, D], MDT, tag="wo")
            chain(nc.sync.dma_start(
                out=wo_sb, in_=wo.rearrange("(g p) n -> p g n", p=128)))
            for j in range(1, LB):
                xqb[j] = xload(xqT, j, 512)

            KT = persist.tile([128, CT, LkP], MDT, tag="kt")
            QT = persist.tile([128, CT, L], MDT, tag="qt")
            ot_sb = persist.tile([128, CT, L], MDT, tag="ot")

            def kproj_ops(j, o=0, wdt=None, xt=None):
                if wdt is None:
                    wdt = kw[j] - o
                ops = []
                for g in range(CT):
                    ps = ps_tile(name="kps")
                    for dc in range(DC):
                        def op_mm(ps=ps, j=j, g=g, dc=dc, o=o, wdt=wdt,
                                  xt=xt):
                            rhs = (xt[:, dc, :wdt] if xt is not None
                                   else xkb[j][:, dc, o:o + wdt])
                            nc.tensor.matmul(
                                ps[:, :wdt],
                                lhsT=w_sb["k"][:, dc, ts(g, 128)],
                                rhs=rhs,
                                start=(dc == 0), stop=(dc == DC - 1))
                        ops.append(op_mm)

                    def op_ev(ps=ps, j=j, g=g, o=o, wdt=wdt):
                        dst = KT[:, g, j * 512 + o:j * 512 + o + wdt]
                        if no_bias:
                            nc.vector.tensor_copy(out=dst, in_=ps[:, :wdt])
                        else:
                            nc.vector.tensor_scalar_add(
                                dst, ps[:, :wdt], b_sb["k"][:, g:g + 1])
                    ops.append(op_ev)
                return ops

            def vproj_ops(c0, c1):
                ops = []
                for c in range(c0, c1):
                    j, o = c // 4, (c % 4) * 128
                    ps = ps_tile(name="vps")
                    if not no_bias:
                        ops.append(lambda ps=ps: nc.tensor.matmul(
                            ps[:, :C], lhsT=ones_sb, rhs=bvr_sb,
                            start=True, stop=False))
                    for dc in range(DC):
                        ops.append(lambda ps=ps, j=j, o=o, dc=dc:
                                   nc.tensor.matmul(
                                       ps[:, :C],
                                       lhsT=xvb[j][:, dc, o:o + 128],
                                       rhs=w_sb["v"][:, dc, :],
                                       start=(no_bias and dc == 0),
                                       stop=(dc == DC - 1)))
                    ops.append(lambda ps=ps, c=c: nc.vector.tensor_copy(
                        out=v_all[:, c, :, 0:DK],
                        in_=ps[:, :C].rearrange("p (h d) -> p h d", h=HPC)))
                return ops

            def qproj_ops(lb):
                ops = []
                for g in range(CT):
                    ps = ps_tile(name="qps")
                    for dc in range(DC):
                        ops.append(lambda ps=ps, lb=lb, g=g, dc=dc:
                                   nc.tensor.matmul(
                                       ps,
                                       lhsT=w_sb["q"][:, dc, ts(g, 128)],
                                       rhs=xqb[lb][:, dc, :],
                                       start=(dc == 0), stop=(dc == DC - 1)))
                    def op_qe(ps=ps, lb=lb, g=g):
                        if no_bias:
                            nc.vector.tensor_copy(
                                out=QT[:, g, ts(lb, 512)], in_=ps)
                        else:
                            nc.vector.tensor_scalar_add(
                                QT[:, g, ts(lb, 512)], ps,
                                b_sb["q"][:, g:g + 1])
                    ops.append(op_qe)
                return ops

            # softmax normalization of bank lb (runs as background ops in
            # bank lb+1): recip (DVE) -> ones-matmul bcast (PE) -> rep evac
            # (DVE) -> multiply (DVE, all-bf16 SBUF)
            def norm_ops(lb, otc, rcs, heads, use_pool=False):
                # 1/denom broadcast across 64 partitions: via the idle
                # GPSIMD engine when latency is hidden (norm runs a wave
                # or bank later), via a PE ones-matmul on the critical
                # epilogue path
                ops = []
                reps = {}
                if use_pool:
                    for h in heads:
                        def op_pb(h=h):
                            rep = normp.tile([64, 512], MDT, tag="rep")
                            nc.gpsimd.partition_broadcast(rep, rcs[h])
                            rcs[h] = rep
                        ops.append(op_pb)
                else:
                    for h in heads:
                        def op_bc(h=h):
                            ps_rep = ps_tile(name="ps_rep")
                            nc.tensor.matmul(ps_rep[:DK, :],
                                             lhsT=ones_sb[:, :DK],
                                             rhs=rcs[h], start=True,
                                             stop=True)
                            reps[h] = ps_rep
                        ops.append(op_bc)
                    for h in heads:
                        def op_rep(h=h):
                            rep = normp.tile([64, 512], MDT, tag="rep")
                            nc.vector.tensor_copy(out=rep,
                                                  in_=reps[h][:DK, :])
                            rcs[h] = rep
                        ops.append(op_rep)
                for h in heads:
                    def op_mul(h=h, lb=lb):
                        g, po = h // 2, 64 * (h % 2)
                        nc.vector.tensor_mul(
                            ot_sb[po:po + DK, g, ts(lb, 512)],
                            otc[h][0:DK, :], rcs[h])
                    ops.append(op_mul)
                return ops

            def oproj_ops(lb):
                ops = []
                for tt in range(4):
                    t = lb * 4 + tt
                    for half in range(2):
                        wps = ps_tile(name="wps")
                        for g in range(CT):
                            ops.append(lambda wps=wps, t=t, half=half, g=g:
                                       nc.tensor.matmul(
                                           wps,
                                           lhsT=ot_sb[:, g, ts(t, 128)],
                                           rhs=wo_sb[:, g, ts(half, 512)],
                                           start=(g == 0),
                                           stop=(g == CT - 1)))

                        def op_stage(wps=wps, t=t, half=half):
                            og = ostgp.tile([128, 512], MDT, tag="os",
                                            name="ostg")
                            nc.vector.tensor_copy(out=og, in_=wps)
                            od = nc.sync.dma_start(
                                out=out[ts(t, 128), ts(half, 512)], in_=og)
                            add_dep_helper(od.ins, last_dma[0].ins,
                                           sync=False, reason="odma-order")
                            last_dma[0] = od
                        ops.append(op_stage)
                return ops

            # ---- foreground prologue: K(j0), Q(0), V(c<VF) ----
            for t_, o_, w_ in xk_seg:
                for op in kproj_ops(0, o_, w_, xt=t_):
                    op()
            for op in qproj_ops(0):
                op()
            for op in vproj_ops(0, VF):
                op()

            # ---- per-Lq-bank software pipeline ----
            # Each bank runs its 4 heads as two g-aligned waves of 2, so
            # only 2 psum banks are held per wave and 6 rotate freely —
            # otherwise every slot turnover is metered by ACT's serial
            # exp pace and PE collects ~180ns stalls per c-iteration.
            norm_pend = None   # norm ops of bank lb-1's second wave
            for lb in range(LB):
                bgw = [[], []]
                if lb == 0:
                    # remaining V tiles and K blocks (wave 0, DMA-paced,
                    # deadline-interleaved: attn@V needs v_all tile c in
                    # slice c; scores need KT block j by slice 4j)
                    ki = 1
                    for i, c in enumerate(range(VF, LTk)):
                        bgw[0] += vproj_ops(c, c + 1)
                        if i % 3 == 2 and ki < KB:
                            bgw[0] += kproj_ops(ki)
                            ki += 1
                    while ki < KB:
                        bgw[0] += kproj_ops(ki)
                        ki += 1
                else:
                    # wave 0: next-bank Q proj (g0 here, g1 in wave 1 —
                    # each wave's scores only read its own g; the QT evac
                    # must not sit behind norm/ostg copies in the in-order
                    # DVE stream), then the rest of bank lb-1's normalize
                    # and half its output projection; wave 1 finishes it
                    op_prev = oproj_ops(lb - 1)
                    if lb + 1 < LB:
                        qp = qproj_ops(lb + 1)
                        bgw[0] += qp[:len(qp) // 2]
                        bgw[1] += qp[len(qp) // 2:]
                    bgw[0] += norm_pend
                    bgw[0] += op_prev[:len(op_prev) // 2]
                    bgw[1] += op_prev[len(op_prev) // 2:]

                otc, rcs = [None] * HPC, [None] * HPC
                for wave in range(2):
                    g = wave
                    bg = bgw[wave]
                    if lb == 0 and wave == 1 and LB > 1:
                        bg += qproj_ops(1)
                    # this bank's first-wave normalize runs in its second
                    # wave; the second wave's normalize goes to bank lb+1
                    if wave == 1:
                        bg += norm_ops(lb, otc, rcs, (0, 1), use_pool=True)
                    takes = [-(-len(bg) // LTk)] * LTk
                    ot_ps = [ps_tile(name=f"ot_{lb}_{g}_{hh}")
                             for hh in range(2)]
                    for c in range(LTk):
                        p_ts = []
                        for hh in range(2):
                            po = 64 * hh
                            s_ps = ps_tile(name="s_ps")
                            nc.tensor.matmul(
                                s_ps,
                                lhsT=KT[po:po + DK, g, ts(c, 128)],
                                rhs=QT[po:po + DK, g, ts(lb, 512)],
                                start=True, stop=True)
                            p_t = work.tile([128, 512], MDT, tag="p")
                            nc.scalar.activation(
                                p_t, s_ps, mybir.ActivationFunctionType.Exp,
                                bias=mb_sb[:, c:c + 1], scale=1.0 / SCALE)
                            p_ts.append(p_t)
                        take = len(bg) if c == LTk - 1 else takes[c]
                        for op in bg[:take]:
                            op()
                        bg = bg[take:]
                        for hh in range(2):
                            h = 2 * wave + hh
                            nc.tensor.matmul(
                                ot_ps[hh][:DK + 1, :],
                                lhsT=v_all[:, c, h, :],
                                rhs=p_ts[hh],
                                start=(c == 0), stop=(c == LTk - 1))
                            if c == LTk - 1:
                                # evacuate + free the psum bank (ACT/DVE
                                # split drains in parallel), then kick the
                                # denominator reciprocal eagerly
                                oc = work.tile([65, 512], MDT, tag="otc",
                                               name="otc")
                                nc.vector.tensor_copy(
                                    out=oc, in_=ot_ps[hh][:DK + 1, :])
                                otc[h] = oc
                                rc = normp.tile([1, 512], MDT, tag="rc")
                                with nc.allow_low_precision(
                                        reason="bf16 softmax denominator "
                                               "recip; |denom|>=1"):
                                    nc.vector.reciprocal(
                                        rc, oc[DK:DK + 1, :])
                                rcs[h] = rc

                norm_pend = norm_ops(lb, otc, rcs, (2, 3),
                     use_pool=True)

            # ---- epilogue: normalize + project the last bank ----
            # the g0 halves of the first output tiles depend only on the
            # already-normalized first wave, so they overlap the final
            # norm chain instead of serializing behind it
            op3 = oproj_ops(LB - 1)   # 8 pairs x (g0 mm, g1 mm, stage)
            early = 5
            for p in range(early):
                op3[p * 3]()
            for op in norm_pend:
                op()
            for p in range(early):
                op3[p * 3 + 1]()
                op3[p * 3 + 2]()
            for op in op3[early * 3:]:
                op()

    nc.compile()
    _strip_implied_dma_ring_waits(nc)
    return nc


def _strip_implied_dma_ring_waits(nc):
    """Drop DMA ring-semaphore waits implied by a compute-engine wait on the
    same descriptor (DMA descriptors carry a single hardware sync-wait).
    Applied to the x-block loads and output-staging stores, whose only
    DMA-semaphore deps are WAW/WAR-release edges already covered by the
    readers' engine semaphore."""
    import concourse.mybir as _mb
    for ins in nc.inst_map.values():
        if type(ins).__name__ != "InstDMACopy":
            continue
        if not ins.outs:
            continue
        memref = getattr(ins.outs[0], "memref", "") or ""
        src_ref = getattr(ins.ins[0], "memref", "") if ins.ins else ""
        if not (memref.startswith("xb") or (src_ref or "").startswith("ostg")):
            continue
        si = ins.sync_info
        if not si or not si.on_wait or len(si.on_wait) < 2:
            continue
        eng = [w_ for w_ in si.on_wait
               if not (w_.ant_name or "").startswith(("DMAHW", "DMASW"))]
        if not eng:
            continue
        ins.sync_info = _mb.SyncInfo(on_wait=eng, on_update=list(si.on_update))


def make_in_maps(query, key, value, mask, Wq, bq, Wk, bk, Wv, bv, Wo, bo,
                 LkP, mm_dtype=mybir.dt.bfloat16, no_bias=False):
    """Host-side sharding + key compaction: per-core input dicts."""
    LTk = LkP // 128
    mdt = mybir.dt.np(mm_dtype)
    in_maps = []
    xTs, mbs = {}, {}
    for b in range(B):
        keep = np.flatnonzero(~mask[b, 0])
        n = len(keep)
        xkc = np.zeros((D_MODEL, LkP), mdt)
        xvc = np.zeros((D_MODEL, LkP), mdt)
        xkc[:, :n] = key[b].T[:, keep].astype(mdt)
        xvc[:, :n] = value[b].T[:, keep].astype(mdt)
        xTs[b] = {
            "q": np.ascontiguousarray(query[b].T.astype(mdt)),
            "k": xkc,
            "v": xvc,
        }
        mbf = np.full(LkP, np.float32(MASK_BIAS))
        mbf[:n] = 0.0
        mbs[b] = np.ascontiguousarray(mbf.reshape(LTk, 128).T)
    for c in range(N_CORES):
        b, hg = divmod(c, N_CORES // B)
        sl = slice(hg * C, (hg + 1) * C)
        in_maps.append({
            "xqT": xTs[b]["q"], "xkT": xTs[b]["k"], "xvT": xTs[b]["v"],
            "wq": np.ascontiguousarray(Wq[:, sl].astype(mdt)),
            "wk": np.ascontiguousarray(Wk[:, sl].astype(mdt)),
            "wv": np.ascontiguousarray(Wv[:, sl].astype(mdt)),
            "wo": np.ascontiguousarray(Wo[sl, :].astype(mdt)),
            "mb": mbs[b],
        })
        if not no_bias:
            in_maps[-1].update({
                "bq": np.ascontiguousarray(bq[sl].astype(np.float32)),
                "bk": np.ascontiguousarray(bk[sl].astype(np.float32)),
                "bvr": np.ascontiguousarray(bv[sl].astype(mdt)[None, :]),
            })
    return in_maps


_NC_CACHE = {}


def _get_nc(LkP, mm_dtype=mybir.dt.bfloat16, no_bias=False):
    key = (str(mm_dtype), LkP, no_bias)
    if key not in _NC_CACHE:
        _NC_CACHE[key] = build_nc(LkP, mm_dtype=mm_dtype, no_bias=no_bias)
    return _NC_CACHE[key]


def run(inputs, mm_dtype=mybir.dt.bfloat16, trace=False):
    """Run on 8 cores; returns (full_output, BassKernelResults)."""
    inputs = {k: np.asarray(v) for k, v in inputs.items()}
    mask = inputs["mask"]
    counts = [int((~mask[b, 0]).sum()) for b in range(B)]
    LkP = max(128, 128 * int(math.ceil(max(counts) / 128.0)))
    no_bias = not (np.any(inputs["bq"]) or np.any(inputs["bk"])
                   or np.any(inputs["bv"]))
    nc = _get_nc(LkP, mm_dtype, no_bias)
    in_maps = make_in_maps(**inputs, LkP=LkP, mm_dtype=mm_dtype,
                           no_bias=no_bias)
    res = run_bass_kernel_spmd(nc, in_maps, list(range(N_CORES)), trace=trace)
    groups_per_batch = N_CORES // B
    out = np.zeros((B, L, D_MODEL), np.float32)
    for b in range(B):
        acc = np.zeros((L, D_MODEL), np.float32)
        if counts[b] > 0:
            for hg in range(groups_per_batch):
                acc += np.asarray(
                    res.results[b * groups_per_batch + hg]["partial"]
                ).astype(np.float32)
        out[b] = acc + inputs["bo"][None, :]
    return out, res


def kernel(**inputs) -> np.ndarray:
    out, _ = run(inputs)
    return out
